# revision 46
# baseline (speedup 1.0000x reference)
"""NeuralSort P_hat @ scores kernel for Trainium2 (8 NeuronCores, data-parallel).

Math per batch row s[0:N], N=1024:
  r_j   = sum_k |s_j - s_k|
  a_i   = N + 1 - 2*(i+1) = 1023 - 2i
  t_ij  = a_i s_j - r_j
  out_i = softmax_j(t_i) . s

Design notes (measured on HW; baseline 344us -> ~181us):
- Near-zero DMA traffic: all operand layouts built on-chip with PE
  transposes (v1 spent 152us of sync-queue DIRECT2D triggers + 9k
  4-byte gather descriptors).
- Compute APs may start only at partitions {0,32,64,96}; every
  cross-partition placement here is either a PE transpose (to base 0)
  or a DMA (unrestricted). DMA APs use plain tile slices only --
  multi-level partition-strided APs silently overrun rows.
- tB and tAn matmuls run ROW-TILED (tile_position=(32*rg,0), 32x128 PE
  tiles) 4-way round-robin over SBUF quadrant replicas of the 5 operand
  rows; replicas maintained by small SBUF->SBUF DMAs. Col tiling
  (nonzero dst quadrant) is rejected by this compiler.
- s3's inner loop is software-pipelined (acc(jt-1) issues after
  tB(jt)) so the in-order PE stream never waits on exp; batch-level
  pipeline is depth 2 (whole prep chain hides under prior batch's s3).
- rowsum passes split DVE(5)/ACT(3); Pool measured ~6x slower on bulk
  tiles and only runs iota/affine_select + a few replica DMAs.

Per core (BPC=8 batches):
  S0  s_bcast row broadcast (DMA, 128 desc)
  S1  rowsum r_j: 8x [128,1024] |diff|-accumulate passes (DVE 2-pass
      with f32 scratch / ACT fused bias-abs-accum 1-pass)
  S1b nr hi/lo splits (col form) + 8 PE transposes -> lhsT5 rows
      (s_hi, nr_hi, s_lo, nr_lo | ones) + quadrant replica DMAs
  S2  anchored row-max bound: f(a)=max_j(a s_j - r_j) is convex; K=2
      row-tiled anchor matmuls at 128 anchors (i=8g) + negate-max
      reduce, PE transpose col->row, Lipschitz pad, neighbor-min with
      x8 stride-0 expansion -> nm row -> rhs5 row 4 (4 quadrant DMAs)
  S3  per jt: K=5 fp32r row-tiled t-matmul [128,1024] (rhs5 =
      (a,1,a,1,nm)), ACT exp -> e (f32r), K=128 reduce matmul -> acc
      (numer_hi, numer_lo, denom); ACT drains acc -> DMA out
Final (hi+lo)/denom division happens on host during unshard (65k flops).

f32r (~12-bit mantissa) operands are hi/lo split where >12 significant
bits matter (s, r); products of split terms are exact and reassemble in
fp32 PSUM. The max-shift M_i needs no lo part: an M error only scales
numer and denom by the same factor, which cancels in the division.
"""

import numpy as np
from contextlib import ExitStack

try:
    import concourse.bass as bass
except ImportError:
    import sys
    sys.path.insert(0, "/opt/trn_rl_repo")
    import concourse.bass as bass
import concourse.tile as tile
from concourse import bacc, mybir
from concourse.bass_utils import run_bass_kernel_spmd

B, N = 64, 1024
NCORES = 8
BPC = B // NCORES      # batches per core
P = 128                # partitions
NT = N // P            # 8 tiles of 128
H = N // 2
F32 = mybir.dt.float32
F32R = mybir.dt.float32r
F16 = mybir.dt.float16
BF16 = mybir.dt.bfloat16
I32 = mybir.dt.int32
AX = mybir.AxisListType
OP = mybir.AluOpType
ACT = mybir.ActivationFunctionType

# rowsum pass assignment per jt: D=DVE 2-pass, A=ACT fused 1-pass.
# Pool is ~6x slower than DVE on bulk tiles (measured) -- bulk work
# never goes there. During pipeline fill (first batches) ACT has no exp
# work yet, so it takes more tiles to shorten the critical path.
RS_STEADY = ("D", "A", "D", "D", "A", "D", "A", "D")
RS_FILL = ("D", "A", "A", "D", "A", "A", "D", "A")
PAD_SLACK = -40.0      # Lipschitz pad for the anchor tail slot


def rep8(base):
    # [1, 128] slice read as [1, 128, 8] with stride-0 inner repeat
    return bass.AP(tensor=base.tensor, offset=base.offset,
                   ap=list(base.ap) + [[0, 8]])


def build_kernel():
    nc = bacc.Bacc("TRN2", target_bir_lowering=False, debug=False)

    scores = nc.dram_tensor("scores", [BPC, N], F32, kind="ExternalInput").ap()
    out3 = nc.dram_tensor("out3", [3, BPC, N], F32, kind="ExternalOutput").ap()

    with tile.TileContext(nc) as tc, ExitStack() as ctx:
        const = ctx.enter_context(tc.tile_pool(name="const", bufs=1))
        perb = ctx.enter_context(tc.tile_pool(name="perb", bufs=2))
        bigs = ctx.enter_context(tc.tile_pool(name="bigs", bufs=2))
        epool = ctx.enter_context(tc.tile_pool(name="epool", bufs=4))
        ps_tb = ctx.enter_context(tc.tile_pool(name="ps_tb", bufs=2, space="PSUM"))
        ps_acc = ctx.enter_context(tc.tile_pool(name="ps_acc", bufs=1, space="PSUM"))
        ps_tan = ctx.enter_context(tc.tile_pool(name="ps_tan", bufs=1, space="PSUM"))
        ps_sm = ctx.enter_context(tc.tile_pool(name="ps_sm", bufs=1, space="PSUM"))

        # scores + batch-0/1 broadcasts first: they gate the prologue and
        # rowsum(0); the ~30 setup DMAs queue behind them on sync.
        scores_t = const.tile([BPC, N], F32)
        nc.sync.dma_start(out=scores_t, in_=scores)
        state = {}

        def s0(bi):
            srow = scores[bi:bi + 1, :]
            s_bcast = bigs.tile([P, N], F32, tag="sb")
            nc.sync.dma_start(out=s_bcast, in_=bass.AP(
                tensor=srow.tensor, offset=srow.offset, ap=[[0, P], [1, N]]))
            state[bi] = dict(s_bcast=s_bcast)

        s0(0)
        s0(1)

        # ---- constants (all on-chip; no const inputs) ----
        ones_f = const.tile([P, P], F32)
        nc.vector.memset(ones_f, 1.0)
        id128f = const.tile([P, P], F32)
        nc.gpsimd.affine_select(out=id128f, in_=ones_f, pattern=[[-1, P]],
                                compare_op=OP.is_equal, fill=0.0,
                                base=0, channel_multiplier=1)
        id128r = const.tile([P, P], F32R)
        nc.vector.tensor_copy(id128r, id128f)
        id8f = id128f[0:8, 0:8]

        a_i32 = const.tile([1, N], I32)
        nc.gpsimd.iota(a_i32, pattern=[[-2, N]], base=N - 1, channel_multiplier=0)
        a_row = const.tile([1, N], F32R)
        nc.vector.tensor_copy(a_row, a_i32)
        anch_i32 = const.tile([1, P], I32)
        nc.gpsimd.iota(anch_i32, pattern=[[-16, P]], base=N - 1, channel_multiplier=0)
        anch_row = const.tile([1, P], F32R)
        nc.vector.tensor_copy(anch_row, anch_i32)
        ones_row_f = const.tile([1, N], F32)
        nc.vector.memset(ones_row_f, 1.0)
        ones_row = const.tile([1, N], F32R)
        nc.vector.tensor_copy(ones_row, ones_row_f)
        ones_cols = const.tile([P, NT * 8], F32)
        nc.vector.memset(ones_cols, 1.0)

        def quad_rows(t, row):
            pitch = t.ap[0][0]
            return bass.AP(tensor=t.tensor, offset=t.offset + row * pitch,
                           ap=[[32 * pitch, 4], [1, N]])

        def row_at(t, row, n, nrows=1, rowstep=32):
            pitch = t.ap[0][0]
            return bass.AP(tensor=t.tensor, offset=t.offset + row * pitch,
                           ap=[[rowstep * pitch, nrows], [1, n]])

        # a_anch rows (a_{8g}, 1) replicated at SBUF quadrants for
        # row-tiled tAn matmuls
        a_anch = const.tile([98, P], F32R)
        nc.sync.dma_start(out=a_anch[0:1, :], in_=anch_row)
        nc.sync.dma_start(out=a_anch[1:2, :], in_=ones_row[0:1, 0:P])
        nc.sync.dma_start(out=a_anch[32:34, :], in_=a_anch[0:2, :])
        nc.sync.dma_start(out=a_anch[64:66, :], in_=a_anch[0:2, :])
        nc.sync.dma_start(out=a_anch[96:98, :], in_=a_anch[0:2, :])

        # operand tiles, manually rotated 4 deep, with replicas of the
        # 5 operand rows at SBUF quadrants {0,32,64,96} so tB/tAn matmuls
        # can run row-tiled (32x128 PE tiles) 4-way concurrently.
        lhsT5_tiles = [const.tile([101, N], F32R, tag=f"lhsT5_{q}", name=f"lhsT5_{q}")
                       for q in range(4)]
        rhs5_tiles = [const.tile([101, N], F32R, tag=f"rhs5_{q}", name=f"rhs5_{q}")
                      for q in range(4)]

        for q in range(4):
            eng = nc.sync if q < 2 else nc.gpsimd
            rt = rhs5_tiles[q]
            eng.dma_start(out=rt[0:1, :], in_=a_row)
            eng.dma_start(out=rt[1:2, :], in_=ones_row)
            eng.dma_start(out=rt[2:3, :], in_=a_row)
            eng.dma_start(out=rt[3:4, :], in_=ones_row)
            for rg in (32, 64, 96):
                eng.dma_start(out=rt[rg:rg + 4, :], in_=rt[0:4, :])
            lt = lhsT5_tiles[q]
            for rg in (4, 36, 68, 100):
                eng.dma_start(out=lt[rg:rg + 1, :], in_=ones_row)

        # ---- prologue: column forms via PE transpose ----
        scols_ps = ps_sm.tile([P, NT * 8], F32, tag="small", name="scols_ps")
        for c in range(NT):
            nc.tensor.transpose(scols_ps[:, c * 8:(c + 1) * 8],
                                scores_t[:, c * P:(c + 1) * P], id8f)
        # b-major column forms: X[p, b, jt] so per-batch slices are
        # contiguous [128, NT] (strided micro-ops measured 5-30x slower)
        scols = const.tile([P, 8, NT], F32)
        nc.vector.tensor_copy(scols.rearrange("p b t -> p t b"),
                              scols_ps.rearrange("p (t b) -> p t b", b=8))
        nscols = const.tile([P, 8, NT], F32)
        nc.vector.tensor_scalar_mul(nscols, scols, -1.0)
        shi_cols = const.tile([P, 8, NT], F32R)
        nc.vector.tensor_copy(shi_cols, scols)
        slo_cols = const.tile([P, 8, NT], F32R)
        nc.vector.tensor_sub(slo_cols, scols, shi_cols.bitcast(F32))
        # se_all[p, b, jt, c]: (s_hi, s_lo, 1) columns, contiguous per (b, jt)
        se_all = const.tile([P, 8, NT, 3], F32R)
        se_v = se_all.rearrange("p b t c -> p c b t")  # write views [P,8,NT]
        nc.vector.tensor_copy(se_v[:, 0], shi_cols)
        nc.vector.tensor_copy(se_v[:, 1], slo_cols)
        nc.vector.tensor_copy(se_v[:, 2], ones_cols.rearrange("p (b t) -> p b t", t=NT))

        def s1(bi):
            st = state[bi]
            s_bcast = st["s_bcast"]
            r_cols = perb.tile([P, NT], F32, tag="rcols")
            assign = RS_FILL if bi < 3 else RS_STEADY
            for jt in range(NT):
                if assign[jt] == "A":
                    scr = bigs.tile([P, N], F32, tag="scr_a", bufs=1)
                    nc.scalar.activation(
                        out=scr, in_=s_bcast, func=ACT.Abs,
                        bias=nscols[:, bi, jt:jt + 1], scale=1.0,
                        accum_out=r_cols[:, jt:jt + 1])
                    continue
                # DVE 2-pass: f32 subtract -> scratch, abs-add reduce
                scr = bigs.tile([P, N], F32, tag="scr_d")
                nc.vector.tensor_scalar(
                    out=scr, in0=s_bcast, scalar1=scols[:, bi, jt:jt + 1],
                    scalar2=None, op0=OP.subtract, op1=OP.bypass)
                nc.vector.tensor_reduce(
                    out=r_cols[:, jt:jt + 1], in_=scr, axis=AX.X,
                    op=OP.add, apply_absolute_value=True)
            st["r_cols"] = r_cols

        def s1b(bi):
            st = state[bi]
            r_cols = st["r_cols"]
            q = bi % 4
            # contiguous computes first, then interleave-casts into cols4
            nrhi_c = perb.tile([P, NT], F32R, tag="nrhi")
            nc.vector.tensor_scalar_mul(nrhi_c, r_cols, -1.0)
            nrlo_c = perb.tile([P, NT], F32R, tag="nrlo")
            nc.vector.scalar_tensor_tensor(
                out=nrlo_c, in0=r_cols, scalar=-1.0,
                in1=nrhi_c.bitcast(F32), op0=OP.mult, op1=OP.subtract)
            # cols4[p, jt, c] = (s_hi, nr_hi, s_lo, nr_lo)
            cols4 = perb.tile([P, NT, 4], F32R, tag="cols4")
            nc.vector.tensor_copy(cols4[:, :, 0], shi_cols[:, bi, :])
            nc.vector.tensor_copy(cols4[:, :, 1], nrhi_c)
            nc.vector.tensor_copy(cols4[:, :, 2], slo_cols[:, bi, :])
            nc.vector.tensor_copy(cols4[:, :, 3], nrlo_c)
            lhsT5 = lhsT5_tiles[q]
            for half in range(2):
                l5h = ps_sm.tile([4, H], F32R, tag="small", name="l5h")
                for j in range(4):
                    jt = half * 4 + j
                    nc.tensor.transpose(l5h[:, j * P:(j + 1) * P],
                                        cols4[:, jt, :], id128r)
                nc.vector.tensor_copy(lhsT5[0:4, half * H:(half + 1) * H], l5h)
            nc.gpsimd.dma_start(out=lhsT5[32:36, :], in_=lhsT5[0:4, :])
            nc.gpsimd.dma_start(out=lhsT5[64:68, :], in_=lhsT5[0:4, :])
            nc.gpsimd.dma_start(out=lhsT5[96:100, :], in_=lhsT5[0:4, :])
            st["lhsT5"] = lhsT5

        def s2(bi):
            st = state[bi]
            lhsT5 = st["lhsT5"]
            q = bi % 4
            # f(a) = max_j (a s_j - r_j) is convex in a; anchors at i=8g.
            # bound M_i = max(f(a_g), f(a_{g+1})) >= f(a_i), slack <= ~40.
            nmA2 = perb.tile([P, 2], F32, tag="nma2")
            for h in range(2):
                tAn = ps_tan.tile([P, H], F32, tag="tan", name="tAn")
                nc.tensor.matmul(tAn, a_anch[32 * h:32 * h + 2, :],
                                 lhsT5[32 * h:32 * h + 2, h * H:(h + 1) * H],
                                 start=True, stop=True,
                                 tile_position=(32 * h, 0))
                nc.vector.tensor_reduce(
                    out=nmA2[:, h:h + 1], in_=tAn, axis=AX.X,
                    op=OP.max, negate=True)
            nmA = perb.tile([P, 1], F32, tag="nma")
            nc.vector.tensor_tensor(out=nmA, in0=nmA2[:, 0:1], in1=nmA2[:, 1:2],
                                    op=OP.min)
            nmrow_ps = ps_sm.tile([1, P], F32, tag="small", name="nmrow_ps")
            nc.tensor.transpose(nmrow_ps, nmA, id128f)
            nmext = perb.tile([1, P + 1], F32, tag="nmext")
            nc.vector.tensor_copy(nmext[0:1, 0:P], nmrow_ps)
            nc.vector.tensor_scalar_add(nmext[0:1, P:P + 1],
                                        nmrow_ps[0:1, P - 1:P], PAD_SLACK)
            # nm_i = min(nmext[i>>3], nmext[(i>>3)+1]), expanded x8 inline
            nm_stage = perb.tile([1, N], F32R, tag="nmstage")
            nc.vector.tensor_tensor(
                out=nm_stage.rearrange("o (g r) -> o g r", r=8),
                in0=rep8(nmext[0:1, 0:P]), in1=rep8(nmext[0:1, 1:P + 1]),
                op=OP.min)
            rt = rhs5_tiles[q]
            nc.sync.dma_start(out=rt[4:5, :], in_=nm_stage)
            nc.sync.dma_start(out=rt[36:37, :], in_=nm_stage)
            nc.gpsimd.dma_start(out=rt[68:69, :], in_=nm_stage)
            nc.gpsimd.dma_start(out=rt[100:101, :], in_=nm_stage)
            st["rhs5"] = rhs5_tiles[q]

        def s3(bi):
            st = state[bi]
            lhsT5, rhs5 = st["lhsT5"], st["rhs5"]
            acc = ps_acc.tile([3, N], F32, tag="acc", name="acc")
            # inner software pipeline: acc(jt-1) issues AFTER tB(jt) so the
            # in-order PE stream never waits on exp(jt) (p-state ramp needs
            # dense PE runs; the serial tB->exp->acc chain cost ~3.6us/jt)
            es = {}

            def emit_acc(jt):
                se = se_all[:, bi, jt, :]
                nc.tensor.matmul(acc[:, 0:H], se, es[jt][:, 0:H],
                                 start=(jt == 0), stop=(jt == NT - 1))
                nc.tensor.matmul(acc[:, H:N], se, es[jt][:, H:N],
                                 start=(jt == 0), stop=(jt == NT - 1))

            for jt in range(NT):
                tB = ps_tb.tile([P, N], F32, tag="tb", name="tB")
                for h in range(2):
                    rg = (2 * jt + h) % 4
                    l5 = lhsT5[32 * rg:32 * rg + 5, jt * P:(jt + 1) * P]
                    r5 = rhs5[32 * rg:32 * rg + 5, h * H:(h + 1) * H]
                    nc.tensor.matmul(tB[:, h * H:(h + 1) * H], l5, r5,
                                     start=True, stop=True,
                                     tile_position=(32 * rg, 0))
                e = epool.tile([P, N], F32R, tag="e", name="e")
                nc.scalar.activation(out=e, in_=tB, func=ACT.Exp)
                es[jt] = e
                if jt >= 1:
                    emit_acc(jt - 1)
            emit_acc(NT - 1)
            # ACT copy: frees acc (bufs=1) promptly -- on DVE this drain
            # queued behind a wave of rowsum work, stalling acc(b+1) on PE
            nd = perb.tile([3, N], F32, tag="nd")
            nc.scalar.copy(out=nd, in_=acc)
            dst = bass.AP(tensor=out3.tensor, offset=out3.offset + bi * N,
                          ap=[[BPC * N, 3], [1, N]])
            nc.sync.dma_start(out=dst, in_=nd)

        # depth-2 software pipeline: batch w's whole prep chain
        # (s1->s1b->s2, ~13us spread over DVE/ACT/PE-small) issues in one
        # wave and hides under s3(w-1)'s ~19us of dense PE work. s3
        # issues FIRST so its cross-engine consumers (nd, next acc) sit
        # at the head of each engine's in-order queue.
        for w in range(BPC + 1):
            if w >= 1:
                s3(w - 1)
            if 2 <= w + 1 < BPC:
                s0(w + 1)
            if w < BPC:
                s1(w)
                s1b(w)
                s2(w)

    nc.compile()
    return nc


_CACHE = {}


def kernel(scores: np.ndarray) -> np.ndarray:
    scores = np.ascontiguousarray(scores, dtype=np.float32)
    assert scores.shape == (B, N)
    if "nc" not in _CACHE:
        _CACHE["nc"] = build_kernel()
    nc = _CACHE["nc"]

    in_maps = [{"scores": scores[c * BPC:(c + 1) * BPC]} for c in range(NCORES)]
    r = run_bass_kernel_spmd(nc, in_maps, core_ids=list(range(NCORES)))
    outs = []
    for c in range(NCORES):
        o3 = r.results[c]["out3"].astype(np.float64)
        outs.append((o3[0] + o3[1]) / o3[2])
    return np.concatenate(outs, axis=0).astype(np.float32)


if __name__ == "__main__":
    x = np.random.randn(B, N).astype(np.float32)
    y = kernel(x)
    print(y.shape, y.dtype)


# revision 48
# speedup vs baseline: 1.1325x; 1.1325x over previous
"""NeuralSort P_hat @ scores kernel for Trainium2 (8 NeuronCores, data-parallel).

Math per batch row s[0:N], N=1024:
  r_j   = sum_k |s_j - s_k|
  a_i   = N + 1 - 2*(i+1) = 1023 - 2i
  t_ij  = a_i s_j - r_j
  out_i = softmax_j(t_i) . s

Design notes (measured on HW; baseline 344us -> ~181us):
- Near-zero DMA traffic: all operand layouts built on-chip with PE
  transposes (v1 spent 152us of sync-queue DIRECT2D triggers + 9k
  4-byte gather descriptors).
- Compute APs may start only at partitions {0,32,64,96}; every
  cross-partition placement here is either a PE transpose (to base 0)
  or a DMA (unrestricted). DMA APs use plain tile slices only --
  multi-level partition-strided APs silently overrun rows.
- tB and tAn matmuls run ROW-TILED (tile_position=(32*rg,0), 32x128 PE
  tiles) 4-way round-robin over SBUF quadrant replicas of the 5 operand
  rows; replicas maintained by small SBUF->SBUF DMAs. Col tiling
  (nonzero dst quadrant) is rejected by this compiler.
- s3's inner loop is software-pipelined (acc(jt-1) issues after
  tB(jt)) so the in-order PE stream never waits on exp; batch-level
  pipeline is depth 2 (whole prep chain hides under prior batch's s3).
- rowsum passes split DVE(5)/ACT(3); Pool measured ~6x slower on bulk
  tiles and only runs iota/affine_select + a few replica DMAs.

Per core (BPC=8 batches):
  S0  s_bcast row broadcast (DMA, 128 desc)
  S1  rowsum r_j: 8x [128,1024] |diff|-accumulate passes (DVE 2-pass
      with f32 scratch / ACT fused bias-abs-accum 1-pass)
  S1b nr hi/lo splits (col form) + 8 PE transposes -> lhsT5 rows
      (s_hi, nr_hi, s_lo, nr_lo | ones) + quadrant replica DMAs
  S2  anchored row-max bound: f(a)=max_j(a s_j - r_j) is convex; K=2
      row-tiled anchor matmuls at 128 anchors (i=8g) + negate-max
      reduce, PE transpose col->row, Lipschitz pad, neighbor-min with
      x8 stride-0 expansion -> nm row -> rhs5 row 4 (4 quadrant DMAs)
  S3  per jt: K=5 fp32r row-tiled t-matmul [128,1024] (rhs5 =
      (a,1,a,1,nm)), ACT exp -> e (f32r), K=128 reduce matmul -> acc
      (numer_hi, numer_lo, denom); ACT drains acc -> DMA out
Final (hi+lo)/denom division happens on host during unshard (65k flops).

f32r (~12-bit mantissa) operands are hi/lo split where >12 significant
bits matter (s, r); products of split terms are exact and reassemble in
fp32 PSUM. The max-shift M_i needs no lo part: an M error only scales
numer and denom by the same factor, which cancels in the division.
"""

import numpy as np
from contextlib import ExitStack

try:
    import concourse.bass as bass
except ImportError:
    import sys
    sys.path.insert(0, "/opt/trn_rl_repo")
    import concourse.bass as bass
import concourse.tile as tile
from concourse import bacc, mybir
from concourse.bass_utils import run_bass_kernel_spmd

B, N = 64, 1024
NCORES = 8
BPC = B // NCORES      # batches per core
P = 128                # partitions
NT = N // P            # 8 tiles of 128
H = N // 2
F32 = mybir.dt.float32
F32R = mybir.dt.float32r
F16 = mybir.dt.float16
BF16 = mybir.dt.bfloat16
I32 = mybir.dt.int32
AX = mybir.AxisListType
OP = mybir.AluOpType
ACT = mybir.ActivationFunctionType

# rowsum pass assignment per jt: D=DVE 2-pass, A=ACT fused 1-pass.
# Pool is ~6x slower than DVE on bulk tiles (measured) -- bulk work
# never goes there. During pipeline fill (first batches) ACT has no exp
# work yet, so it takes more tiles to shorten the critical path.
RS_STEADY = ("D", "A", "D", "D", "A", "D", "A", "D")
RS_FILL = ("D", "A", "A", "D", "A", "A", "D", "A")
PAD_SLACK = -40.0      # Lipschitz pad for the anchor tail slot


def rep8(base):
    # [1, 128] slice read as [1, 128, 8] with stride-0 inner repeat
    return bass.AP(tensor=base.tensor, offset=base.offset,
                   ap=list(base.ap) + [[0, 8]])


def build_kernel():
    nc = bacc.Bacc("TRN2", target_bir_lowering=False, debug=False)

    scores = nc.dram_tensor("scores", [BPC, N], F32, kind="ExternalInput").ap()
    out3 = nc.dram_tensor("out3", [3, BPC, N], F32, kind="ExternalOutput").ap()

    with tile.TileContext(nc) as tc, ExitStack() as ctx:
        const = ctx.enter_context(tc.tile_pool(name="const", bufs=1))
        perb = ctx.enter_context(tc.tile_pool(name="perb", bufs=2))
        bigs = ctx.enter_context(tc.tile_pool(name="bigs", bufs=2))
        epool = ctx.enter_context(tc.tile_pool(name="epool", bufs=4))
        ps_tb = ctx.enter_context(tc.tile_pool(name="ps_tb", bufs=2, space="PSUM"))
        ps_acc = ctx.enter_context(tc.tile_pool(name="ps_acc", bufs=1, space="PSUM"))
        ps_tan = ctx.enter_context(tc.tile_pool(name="ps_tan", bufs=1, space="PSUM"))
        ps_sm = ctx.enter_context(tc.tile_pool(name="ps_sm", bufs=1, space="PSUM"))


        # ---- constants (all on-chip; no const inputs) ----
        ones_f = const.tile([P, P], F32)
        nc.vector.memset(ones_f, 1.0)
        id128f = const.tile([P, P], F32)
        nc.gpsimd.affine_select(out=id128f, in_=ones_f, pattern=[[-1, P]],
                                compare_op=OP.is_equal, fill=0.0,
                                base=0, channel_multiplier=1)
        id128r = const.tile([P, P], F32R)
        nc.vector.tensor_copy(id128r, id128f)
        id8f = id128f[0:8, 0:8]

        a_i32 = const.tile([1, N], I32)
        nc.gpsimd.iota(a_i32, pattern=[[-2, N]], base=N - 1, channel_multiplier=0)
        a_row = const.tile([1, N], F32R)
        nc.vector.tensor_copy(a_row, a_i32)
        anch_i32 = const.tile([1, P], I32)
        nc.gpsimd.iota(anch_i32, pattern=[[-16, P]], base=N - 1, channel_multiplier=0)
        anch_row = const.tile([1, P], F32R)
        nc.vector.tensor_copy(anch_row, anch_i32)
        ones_row_f = const.tile([1, N], F32)
        nc.vector.memset(ones_row_f, 1.0)
        ones_row = const.tile([1, N], F32R)
        nc.vector.tensor_copy(ones_row, ones_row_f)
        ones_cols = const.tile([P, NT * 8], F32)
        nc.vector.memset(ones_cols, 1.0)

        def quad_rows(t, row):
            pitch = t.ap[0][0]
            return bass.AP(tensor=t.tensor, offset=t.offset + row * pitch,
                           ap=[[32 * pitch, 4], [1, N]])

        def row_at(t, row, n, nrows=1, rowstep=32):
            pitch = t.ap[0][0]
            return bass.AP(tensor=t.tensor, offset=t.offset + row * pitch,
                           ap=[[rowstep * pitch, nrows], [1, n]])

        # a_anch rows (a_{8g}, 1) replicated at SBUF quadrants for
        # row-tiled tAn matmuls
        a_anch = const.tile([98, P], F32R)
        nc.sync.dma_start(out=a_anch[0:1, :], in_=anch_row)
        nc.sync.dma_start(out=a_anch[1:2, :], in_=ones_row[0:1, 0:P])
        nc.sync.dma_start(out=a_anch[32:34, :], in_=a_anch[0:2, :])
        nc.sync.dma_start(out=a_anch[64:66, :], in_=a_anch[0:2, :])
        nc.sync.dma_start(out=a_anch[96:98, :], in_=a_anch[0:2, :])

        # operand tiles, manually rotated 4 deep, with replicas of the
        # 5 operand rows at SBUF quadrants {0,32,64,96} so tB/tAn matmuls
        # can run row-tiled (32x128 PE tiles) 4-way concurrently.
        lhsT5_tiles = [const.tile([101, N], F32R, tag=f"lhsT5_{q}", name=f"lhsT5_{q}")
                       for q in range(4)]
        rhs5_tiles = [const.tile([101, N], F32R, tag=f"rhs5_{q}", name=f"rhs5_{q}")
                      for q in range(4)]

        for q in range(4):
            eng = nc.sync if q % 2 == 0 else nc.gpsimd
            rt = rhs5_tiles[q]
            eng.dma_start(out=rt[0:1, :], in_=a_row)
            eng.dma_start(out=rt[1:2, :], in_=ones_row)
            eng.dma_start(out=rt[2:3, :], in_=a_row)
            eng.dma_start(out=rt[3:4, :], in_=ones_row)
            for rg in (32, 64, 96):
                eng.dma_start(out=rt[rg:rg + 4, :], in_=rt[0:4, :])
            lt = lhsT5_tiles[q]
            for rg in (4, 36, 68, 100):
                eng.dma_start(out=lt[rg:rg + 1, :], in_=ones_row)

        # ---- prologue: scores load + column forms via PE transpose ----
        scores_t = const.tile([BPC, N], F32)
        nc.sync.dma_start(out=scores_t, in_=scores)
        scols_ps = ps_sm.tile([P, NT * 8], F32, tag="small", name="scols_ps")
        for c in range(NT):
            nc.tensor.transpose(scols_ps[:, c * 8:(c + 1) * 8],
                                scores_t[:, c * P:(c + 1) * P], id8f)
        # b-major column forms: X[p, b, jt] so per-batch slices are
        # contiguous [128, NT] (strided micro-ops measured 5-30x slower)
        scols = const.tile([P, 8, NT], F32)
        nc.vector.tensor_copy(scols.rearrange("p b t -> p t b"),
                              scols_ps.rearrange("p (t b) -> p t b", b=8))
        nscols = const.tile([P, 8, NT], F32)
        nc.vector.tensor_scalar_mul(nscols, scols, -1.0)
        shi_cols = const.tile([P, 8, NT], F32R)
        nc.vector.tensor_copy(shi_cols, scols)
        slo_cols = const.tile([P, 8, NT], F32R)
        nc.vector.tensor_sub(slo_cols, scols, shi_cols.bitcast(F32))
        # se_all[p, b, jt, c]: (s_hi, s_lo, 1) columns, contiguous per (b, jt)
        se_all = const.tile([P, 8, NT, 3], F32R)
        se_v = se_all.rearrange("p b t c -> p c b t")  # write views [P,8,NT]
        nc.vector.tensor_copy(se_v[:, 0], shi_cols)
        nc.vector.tensor_copy(se_v[:, 1], slo_cols)
        nc.vector.tensor_copy(se_v[:, 2], ones_cols.rearrange("p (b t) -> p b t", t=NT))

        state = {}

        def s0(bi):
            srow = scores[bi:bi + 1, :]
            s_bcast = bigs.tile([P, N], F32, tag="sb")
            nc.sync.dma_start(out=s_bcast, in_=bass.AP(
                tensor=srow.tensor, offset=srow.offset, ap=[[0, P], [1, N]]))
            state[bi] = dict(s_bcast=s_bcast)

        def s1(bi):
            st = state[bi]
            s_bcast = st["s_bcast"]
            r_cols = perb.tile([P, NT], F32, tag="rcols")
            assign = RS_FILL if bi < 3 else RS_STEADY
            for jt in range(NT):
                if assign[jt] == "A":
                    scr = bigs.tile([P, N], F32, tag="scr_a", bufs=1)
                    nc.scalar.activation(
                        out=scr, in_=s_bcast, func=ACT.Abs,
                        bias=nscols[:, bi, jt:jt + 1], scale=1.0,
                        accum_out=r_cols[:, jt:jt + 1])
                    continue
                # DVE 2-pass: f32 subtract -> scratch, abs-add reduce
                scr = bigs.tile([P, N], F32, tag="scr_d")
                nc.vector.tensor_scalar(
                    out=scr, in0=s_bcast, scalar1=scols[:, bi, jt:jt + 1],
                    scalar2=None, op0=OP.subtract, op1=OP.bypass)
                nc.vector.tensor_reduce(
                    out=r_cols[:, jt:jt + 1], in_=scr, axis=AX.X,
                    op=OP.add, apply_absolute_value=True)
            st["r_cols"] = r_cols

        def s1b(bi):
            st = state[bi]
            r_cols = st["r_cols"]
            q = bi % 4
            # contiguous computes first, then interleave-casts into cols4
            nrhi_c = perb.tile([P, NT], F32R, tag="nrhi")
            nc.vector.tensor_scalar_mul(nrhi_c, r_cols, -1.0)
            nrlo_c = perb.tile([P, NT], F32R, tag="nrlo")
            nc.vector.scalar_tensor_tensor(
                out=nrlo_c, in0=r_cols, scalar=-1.0,
                in1=nrhi_c.bitcast(F32), op0=OP.mult, op1=OP.subtract)
            # cols4[p, jt, c] = (s_hi, nr_hi, s_lo, nr_lo)
            cols4 = perb.tile([P, NT, 4], F32R, tag="cols4")
            nc.vector.tensor_copy(cols4[:, :, 0], shi_cols[:, bi, :])
            nc.vector.tensor_copy(cols4[:, :, 1], nrhi_c)
            nc.vector.tensor_copy(cols4[:, :, 2], slo_cols[:, bi, :])
            nc.vector.tensor_copy(cols4[:, :, 3], nrlo_c)
            lhsT5 = lhsT5_tiles[q]
            for half in range(2):
                l5h = ps_sm.tile([4, H], F32R, tag="small", name="l5h")
                for j in range(4):
                    jt = half * 4 + j
                    nc.tensor.transpose(l5h[:, j * P:(j + 1) * P],
                                        cols4[:, jt, :], id128r)
                nc.vector.tensor_copy(lhsT5[0:4, half * H:(half + 1) * H], l5h)
            nc.gpsimd.dma_start(out=lhsT5[32:36, :], in_=lhsT5[0:4, :])
            nc.gpsimd.dma_start(out=lhsT5[64:68, :], in_=lhsT5[0:4, :])
            nc.gpsimd.dma_start(out=lhsT5[96:100, :], in_=lhsT5[0:4, :])
            st["lhsT5"] = lhsT5

        def s2(bi):
            st = state[bi]
            lhsT5 = st["lhsT5"]
            q = bi % 4
            # f(a) = max_j (a s_j - r_j) is convex in a; anchors at i=8g.
            # bound M_i = max(f(a_g), f(a_{g+1})) >= f(a_i), slack <= ~40.
            nmA2 = perb.tile([P, 2], F32, tag="nma2")
            # h0/h1 in different PSUM banks (ps_tan / ps_sm slot) so the two
            # row-tiled anchor matmuls overlap instead of serializing on a
            # WAR against the h0 reduce
            tAn0 = ps_tan.tile([P, H], F32, tag="tan", name="tAn0")
            nc.tensor.matmul(tAn0, a_anch[0:2, :], lhsT5[0:2, 0:H],
                             start=True, stop=True, tile_position=(0, 0))
            tAn1 = ps_sm.tile([P, H], F32, tag="small", name="tAn1")
            nc.tensor.matmul(tAn1, a_anch[32:34, :], lhsT5[32:34, H:N],
                             start=True, stop=True, tile_position=(32, 0))
            nc.vector.tensor_reduce(out=nmA2[:, 0:1], in_=tAn0, axis=AX.X,
                                    op=OP.max, negate=True)
            nc.vector.tensor_reduce(out=nmA2[:, 1:2], in_=tAn1, axis=AX.X,
                                    op=OP.max, negate=True)
            nmA = perb.tile([P, 1], F32, tag="nma")
            nc.vector.tensor_tensor(out=nmA, in0=nmA2[:, 0:1], in1=nmA2[:, 1:2],
                                    op=OP.min)
            nmrow_ps = ps_sm.tile([1, P], F32, tag="small", name="nmrow_ps")
            nc.tensor.transpose(nmrow_ps, nmA, id128f)
            nmext = perb.tile([1, P + 1], F32, tag="nmext")
            nc.vector.tensor_copy(nmext[0:1, 0:P], nmrow_ps)
            nc.vector.tensor_scalar_add(nmext[0:1, P:P + 1],
                                        nmrow_ps[0:1, P - 1:P], PAD_SLACK)
            # nm_i = min(nmext[i>>3], nmext[(i>>3)+1]), expanded x8 inline
            nm_stage = perb.tile([1, N], F32R, tag="nmstage")
            nc.vector.tensor_tensor(
                out=nm_stage.rearrange("o (g r) -> o g r", r=8),
                in0=rep8(nmext[0:1, 0:P]), in1=rep8(nmext[0:1, 1:P + 1]),
                op=OP.min)
            rt = rhs5_tiles[q]
            nc.sync.dma_start(out=rt[4:5, :], in_=nm_stage)
            nc.sync.dma_start(out=rt[36:37, :], in_=nm_stage)
            nc.gpsimd.dma_start(out=rt[68:69, :], in_=nm_stage)
            nc.gpsimd.dma_start(out=rt[100:101, :], in_=nm_stage)
            st["rhs5"] = rhs5_tiles[q]

        def s3(bi):
            st = state[bi]
            lhsT5, rhs5 = st["lhsT5"], st["rhs5"]
            acc = ps_acc.tile([3, N], F32, tag="acc", name="acc")
            # inner software pipeline: acc(jt-1) issues AFTER tB(jt) so the
            # in-order PE stream never waits on exp(jt) (p-state ramp needs
            # dense PE runs; the serial tB->exp->acc chain cost ~3.6us/jt)
            es = {}

            def emit_acc(jt):
                se = se_all[:, bi, jt, :]
                nc.tensor.matmul(acc[:, 0:H], se, es[jt][:, 0:H],
                                 start=(jt == 0), stop=(jt == NT - 1))
                nc.tensor.matmul(acc[:, H:N], se, es[jt][:, H:N],
                                 start=(jt == 0), stop=(jt == NT - 1))

            for jt in range(NT):
                tB = ps_tb.tile([P, N], F32, tag="tb", name="tB")
                for h in range(2):
                    rg = (2 * jt + h) % 4
                    l5 = lhsT5[32 * rg:32 * rg + 5, jt * P:(jt + 1) * P]
                    r5 = rhs5[32 * rg:32 * rg + 5, h * H:(h + 1) * H]
                    nc.tensor.matmul(tB[:, h * H:(h + 1) * H], l5, r5,
                                     start=True, stop=True,
                                     tile_position=(32 * rg, 0))
                e = epool.tile([P, N], F32R, tag="e", name="e")
                nc.scalar.activation(out=e, in_=tB, func=ACT.Exp)
                es[jt] = e
                if jt >= 1:
                    emit_acc(jt - 1)
            emit_acc(NT - 1)
            # ACT copy: frees acc (bufs=1) promptly -- on DVE this drain
            # queued behind a wave of rowsum work, stalling acc(b+1) on PE
            nd = perb.tile([3, N], F32, tag="nd")
            nc.vector.tensor_copy(nd, acc)
            dst = bass.AP(tensor=out3.tensor, offset=out3.offset + bi * N,
                          ap=[[BPC * N, 3], [1, N]])
            nc.sync.dma_start(out=dst, in_=nd)

        # depth-2 software pipeline: batch w's whole prep chain
        # (s1->s1b->s2, ~13us spread over DVE/ACT/PE-small) issues in one
        # wave and hides under s3(w-1)'s ~19us of dense PE work. s3
        # issues FIRST so its cross-engine consumers (nd, next acc) sit
        # at the head of each engine's in-order queue.
        s0(0)
        for w in range(BPC + 1):
            if w >= 1:
                s3(w - 1)
            if w + 1 < BPC:
                s0(w + 1)
            if w < BPC:
                s1(w)
                s1b(w)
                s2(w)

    nc.compile()
    return nc


_CACHE = {}


def kernel(scores: np.ndarray) -> np.ndarray:
    scores = np.ascontiguousarray(scores, dtype=np.float32)
    assert scores.shape == (B, N)
    if "nc" not in _CACHE:
        _CACHE["nc"] = build_kernel()
    nc = _CACHE["nc"]

    in_maps = [{"scores": scores[c * BPC:(c + 1) * BPC]} for c in range(NCORES)]
    r = run_bass_kernel_spmd(nc, in_maps, core_ids=list(range(NCORES)))
    outs = []
    for c in range(NCORES):
        o3 = r.results[c]["out3"].astype(np.float64)
        outs.append((o3[0] + o3[1]) / o3[2])
    return np.concatenate(outs, axis=0).astype(np.float32)


if __name__ == "__main__":
    x = np.random.randn(B, N).astype(np.float32)
    y = kernel(x)
    print(y.shape, y.dtype)


# revision 53
# speedup vs baseline: 1.2057x; 1.0646x over previous
"""NeuralSort P_hat @ scores kernel for Trainium2 (8 NeuronCores, data-parallel).

Math per batch row s[0:N], N=1024:
  r_j   = sum_k |s_j - s_k|
  a_i   = N + 1 - 2*(i+1) = 1023 - 2i
  t_ij  = a_i s_j - r_j
  out_i = softmax_j(t_i) . s

Design notes (measured on HW; baseline 344us -> ~181us):
- Near-zero DMA traffic: all operand layouts built on-chip with PE
  transposes (v1 spent 152us of sync-queue DIRECT2D triggers + 9k
  4-byte gather descriptors).
- Compute APs may start only at partitions {0,32,64,96}; every
  cross-partition placement here is either a PE transpose (to base 0)
  or a DMA (unrestricted). DMA APs use plain tile slices only --
  multi-level partition-strided APs silently overrun rows.
- tB and tAn matmuls run ROW-TILED (tile_position=(32*rg,0), 32x128 PE
  tiles) 4-way round-robin over SBUF quadrant replicas of the 5 operand
  rows; replicas maintained by small SBUF->SBUF DMAs. Col tiling
  (nonzero dst quadrant) is rejected by this compiler.
- s3's inner loop is software-pipelined (acc(jt-1) issues after
  tB(jt)) so the in-order PE stream never waits on exp; batch-level
  pipeline is depth 2 (whole prep chain hides under prior batch's s3).
- rowsum (ReLU form) split DVE(5)/ACT(3); Pool measured ~6x slower on bulk
  tiles and only runs iota/affine_select + a few replica DMAs.

Per core (BPC=8 batches):
  S0  s_bcast row broadcast (DMA, 128 desc)
  S1  rowsum via ReLU identity: r_j = 2*sum_k relu(s_k - s_j) - S + n*s_j;
      the -S term is row-constant (cancels in softmax, dropped) and n*s_j
      folds into a' = a - n. One fused pass per tile: DVE
      scalar_tensor_tensor(sub, max, sum-accum) / ACT Relu bias-accum
  S1b nr hi/lo splits (col form) + 8 PE transposes -> lhsT5 rows
      (s_hi, nr_hi, s_lo, nr_lo | ones) + quadrant replica DMAs
  S2  anchored row-max bound: f(a)=max_j(a s_j - r_j) is convex; K=2
      row-tiled anchor matmuls at 128 anchors (i=8g) + negate-max
      reduce, PE transpose col->row, Lipschitz pad, neighbor-min with
      x8 stride-0 expansion -> nm row -> rhs5 row 4 (4 quadrant DMAs)
  S3  per jt: K=5 fp32r row-tiled t-matmul [128,1024] (rhs5 =
      (a,1,a,1,nm)), ACT exp -> e (f32r), K=128 reduce matmul -> acc
      (numer_hi, numer_lo, denom); ACT drains acc -> DMA out
Final (hi+lo)/denom division happens on host during unshard (65k flops).

f32r (~12-bit mantissa) operands are hi/lo split where >12 significant
bits matter (s, r); products of split terms are exact and reassemble in
fp32 PSUM. The max-shift M_i needs no lo part: an M error only scales
numer and denom by the same factor, which cancels in the division.
"""

import numpy as np
from contextlib import ExitStack

try:
    import concourse.bass as bass
except ImportError:
    import sys
    sys.path.insert(0, "/opt/trn_rl_repo")
    import concourse.bass as bass
import concourse.tile as tile
from concourse import bacc, mybir
from concourse.bass_utils import run_bass_kernel_spmd

B, N = 64, 1024
NCORES = 8
BPC = B // NCORES      # batches per core
P = 128                # partitions
NT = N // P            # 8 tiles of 128
H = N // 2
F32 = mybir.dt.float32
F32R = mybir.dt.float32r
F16 = mybir.dt.float16
BF16 = mybir.dt.bfloat16
I32 = mybir.dt.int32
AX = mybir.AxisListType
OP = mybir.AluOpType
ACT = mybir.ActivationFunctionType

# rowsum pass assignment per jt: D=DVE 2-pass, A=ACT fused 1-pass.
# Pool is ~6x slower than DVE on bulk tiles (measured) -- bulk work
# never goes there. During pipeline fill (first batches) ACT has no exp
# work yet, so it takes more tiles to shorten the critical path.
RS_STEADY = ("D", "A", "D", "D", "A", "D", "A", "D")
RS_FILL = ("D", "A", "A", "D", "A", "A", "D", "A")
PAD_SLACK = -40.0      # Lipschitz pad for the anchor tail slot


def rep8(base):
    # [1, 128] slice read as [1, 128, 8] with stride-0 inner repeat
    return bass.AP(tensor=base.tensor, offset=base.offset,
                   ap=list(base.ap) + [[0, 8]])


def build_kernel():
    nc = bacc.Bacc("TRN2", target_bir_lowering=False, debug=False)

    scores = nc.dram_tensor("scores", [BPC, N], F32, kind="ExternalInput").ap()
    out3 = nc.dram_tensor("out3", [3, BPC, N], F32, kind="ExternalOutput").ap()

    with tile.TileContext(nc) as tc, ExitStack() as ctx:
        const = ctx.enter_context(tc.tile_pool(name="const", bufs=1))
        perb = ctx.enter_context(tc.tile_pool(name="perb", bufs=2))
        bigs = ctx.enter_context(tc.tile_pool(name="bigs", bufs=2))
        epool = ctx.enter_context(tc.tile_pool(name="epool", bufs=4))
        ps_tb = ctx.enter_context(tc.tile_pool(name="ps_tb", bufs=2, space="PSUM"))
        ps_acc = ctx.enter_context(tc.tile_pool(name="ps_acc", bufs=1, space="PSUM"))
        ps_tan = ctx.enter_context(tc.tile_pool(name="ps_tan", bufs=1, space="PSUM"))
        ps_sm = ctx.enter_context(tc.tile_pool(name="ps_sm", bufs=1, space="PSUM"))

        # critical loads first on the sync FIFO: scores + batch-0/1
        # broadcasts gate the whole fill; setup DMAs queue behind them.
        scores_t = const.tile([BPC, N], F32)
        nc.sync.dma_start(out=scores_t, in_=scores)
        state = {}

        def s0(bi):
            srow = scores[bi:bi + 1, :]
            s_bcast = bigs.tile([P, N], F32, tag="sb")
            nc.sync.dma_start(out=s_bcast, in_=bass.AP(
                tensor=srow.tensor, offset=srow.offset, ap=[[0, P], [1, N]]))
            state[bi] = dict(s_bcast=s_bcast)

        s0(0)
        s0(1)

        # ---- constants (all on-chip; no const inputs) ----
        ones_f = const.tile([P, P], F32)
        nc.vector.memset(ones_f, 1.0)
        id128f = const.tile([P, P], F32)
        nc.gpsimd.affine_select(out=id128f, in_=ones_f, pattern=[[-1, P]],
                                compare_op=OP.is_equal, fill=0.0,
                                base=0, channel_multiplier=1)
        id128r = const.tile([P, P], F32R)
        nc.vector.tensor_copy(id128r, id128f)
        id8f = id128f[0:8, 0:8]

        a_i32 = const.tile([1, N], I32)
        nc.gpsimd.iota(a_i32, pattern=[[-2, N]], base=-1, channel_multiplier=0)
        a_row = const.tile([1, N], F32R)
        nc.vector.tensor_copy(a_row, a_i32)
        anch_i32 = const.tile([1, P], I32)
        nc.gpsimd.iota(anch_i32, pattern=[[-16, P]], base=-1, channel_multiplier=0)
        anch_row = const.tile([1, P], F32R)
        nc.vector.tensor_copy(anch_row, anch_i32)
        ones_row_f = const.tile([1, N], F32)
        nc.vector.memset(ones_row_f, 1.0)
        ones_row = const.tile([1, N], F32R)
        nc.vector.tensor_copy(ones_row, ones_row_f)
        ones_cols = const.tile([P, NT * 8], F32)
        nc.vector.memset(ones_cols, 1.0)
        zeros_t = const.tile([P, N], F32)
        nc.vector.memset(zeros_t, 0.0)

        def quad_rows(t, row):
            pitch = t.ap[0][0]
            return bass.AP(tensor=t.tensor, offset=t.offset + row * pitch,
                           ap=[[32 * pitch, 4], [1, N]])

        def row_at(t, row, n, nrows=1, rowstep=32):
            pitch = t.ap[0][0]
            return bass.AP(tensor=t.tensor, offset=t.offset + row * pitch,
                           ap=[[rowstep * pitch, nrows], [1, n]])

        # a_anch rows (a_{8g}, 1) replicated at SBUF quadrants for
        # row-tiled tAn matmuls
        a_anch = const.tile([98, P], F32R)
        nc.sync.dma_start(out=a_anch[0:1, :], in_=anch_row)
        nc.sync.dma_start(out=a_anch[1:2, :], in_=ones_row[0:1, 0:P])
        nc.sync.dma_start(out=a_anch[32:34, :], in_=a_anch[0:2, :])
        nc.sync.dma_start(out=a_anch[64:66, :], in_=a_anch[0:2, :])
        nc.sync.dma_start(out=a_anch[96:98, :], in_=a_anch[0:2, :])

        # operand tiles, manually rotated 4 deep, with replicas of the
        # 5 operand rows at SBUF quadrants {0,32,64,96} so tB/tAn matmuls
        # can run row-tiled (32x128 PE tiles) 4-way concurrently.
        lhsT5_tiles = [const.tile([101, N], F32R, tag=f"lhsT5_{q}", name=f"lhsT5_{q}")
                       for q in range(4)]
        rhs5_tiles = [const.tile([101, N], F32R, tag=f"rhs5_{q}", name=f"rhs5_{q}")
                      for q in range(4)]

        for q in range(4):
            eng = nc.sync if q % 2 == 0 else nc.gpsimd
            rt = rhs5_tiles[q]
            eng.dma_start(out=rt[0:1, :], in_=a_row)
            eng.dma_start(out=rt[1:2, :], in_=ones_row)
            eng.dma_start(out=rt[2:3, :], in_=a_row)
            eng.dma_start(out=rt[3:4, :], in_=ones_row)
            for rg in (32, 64, 96):
                eng.dma_start(out=rt[rg:rg + 4, :], in_=rt[0:4, :])
            lt = lhsT5_tiles[q]
            for rg in (4, 36, 68, 100):
                eng.dma_start(out=lt[rg:rg + 1, :], in_=ones_row)

        # ---- prologue: column forms via PE transpose ----
        scols_ps = ps_sm.tile([P, NT * 8], F32, tag="small", name="scols_ps")
        for c in range(NT):
            nc.tensor.transpose(scols_ps[:, c * 8:(c + 1) * 8],
                                scores_t[:, c * P:(c + 1) * P], id8f)
        # b-major column forms: X[p, b, jt] so per-batch slices are
        # contiguous [128, NT] (strided micro-ops measured 5-30x slower)
        scols = const.tile([P, 8, NT], F32)
        nc.vector.tensor_copy(scols.rearrange("p b t -> p t b"),
                              scols_ps.rearrange("p (t b) -> p t b", b=8))
        nscols = const.tile([P, 8, NT], F32)
        nc.vector.tensor_scalar_mul(nscols, scols, -1.0)
        shi_cols = const.tile([P, 8, NT], F32R)
        nc.vector.tensor_copy(shi_cols, scols)
        slo_cols = const.tile([P, 8, NT], F32R)
        nc.vector.tensor_sub(slo_cols, scols, shi_cols.bitcast(F32))
        # se_all[p, b, jt, c]: (s_hi, s_lo, 1) columns, contiguous per (b, jt)
        se_all = const.tile([P, 8, NT, 3], F32R)
        se_v = se_all.rearrange("p b t c -> p c b t")  # write views [P,8,NT]
        nc.vector.tensor_copy(se_v[:, 0], shi_cols)
        nc.vector.tensor_copy(se_v[:, 1], slo_cols)
        nc.vector.tensor_copy(se_v[:, 2], ones_cols.rearrange("p (b t) -> p b t", t=NT))

        def s1(bi):
            st = state[bi]
            s_bcast = st["s_bcast"]
            r_cols = perb.tile([P, NT], F32, tag="rcols")
            assign = RS_FILL if bi < 3 else RS_STEADY
            for jt in range(NT):
                if assign[jt] == "A":
                    scr = bigs.tile([P, N], F32, tag="scr_a", bufs=1)
                    nc.scalar.activation(
                        out=scr, in_=s_bcast, func=ACT.Relu,
                        bias=nscols[:, bi, jt:jt + 1], scale=1.0,
                        accum_out=r_cols[:, jt:jt + 1])
                    continue
                # DVE fused 1-pass: A_j = sum_k max(s_k - s_j, 0)
                scr = bigs.tile([P, N], F32, tag="scr_d")
                nc.vector.scalar_tensor_tensor(
                    out=scr, in0=s_bcast, scalar=scols[:, bi, jt:jt + 1],
                    in1=zeros_t, op0=OP.subtract, op1=OP.max,
                    accum_out=r_cols[:, jt:jt + 1])
            st["r_cols"] = r_cols

        def s1b(bi):
            st = state[bi]
            r_cols = st["r_cols"]
            q = bi % 4
            # contiguous computes first, then interleave-casts into cols4
            nrhi_c = perb.tile([P, NT], F32R, tag="nrhi")
            nc.vector.tensor_scalar_mul(nrhi_c, r_cols, -2.0)
            nrlo_c = perb.tile([P, NT], F32R, tag="nrlo")
            nc.vector.scalar_tensor_tensor(
                out=nrlo_c, in0=r_cols, scalar=-2.0,
                in1=nrhi_c.bitcast(F32), op0=OP.mult, op1=OP.subtract)
            # cols4[p, jt, c] = (s_hi, nr_hi, s_lo, nr_lo)
            cols4 = perb.tile([P, NT, 4], F32R, tag="cols4")
            nc.vector.tensor_copy(cols4[:, :, 0], shi_cols[:, bi, :])
            nc.vector.tensor_copy(cols4[:, :, 1], nrhi_c)
            nc.vector.tensor_copy(cols4[:, :, 2], slo_cols[:, bi, :])
            nc.vector.tensor_copy(cols4[:, :, 3], nrlo_c)
            lhsT5 = lhsT5_tiles[q]
            for half in range(2):
                l5h = ps_sm.tile([4, H], F32R, tag="small", name="l5h")
                for j in range(4):
                    jt = half * 4 + j
                    nc.tensor.transpose(l5h[:, j * P:(j + 1) * P],
                                        cols4[:, jt, :], id128r)
                nc.vector.tensor_copy(lhsT5[0:4, half * H:(half + 1) * H], l5h)
            nc.gpsimd.dma_start(out=lhsT5[32:36, :], in_=lhsT5[0:4, :])
            nc.gpsimd.dma_start(out=lhsT5[64:68, :], in_=lhsT5[0:4, :])
            nc.gpsimd.dma_start(out=lhsT5[96:100, :], in_=lhsT5[0:4, :])
            st["lhsT5"] = lhsT5

        def s2(bi):
            st = state[bi]
            lhsT5 = st["lhsT5"]
            q = bi % 4
            # f(a) = max_j (a s_j - r_j) is convex in a; anchors at i=8g.
            # bound M_i = max(f(a_g), f(a_{g+1})) >= f(a_i), slack <= ~40.
            nmA2 = perb.tile([P, 2], F32, tag="nma2")
            for h in range(2):
                tAn = ps_tan.tile([P, H], F32, tag="tan", name="tAn")
                nc.tensor.matmul(tAn, a_anch[32 * h:32 * h + 2, :],
                                 lhsT5[32 * h:32 * h + 2, h * H:(h + 1) * H],
                                 start=True, stop=True,
                                 tile_position=(32 * h, 0))
                nc.vector.tensor_reduce(
                    out=nmA2[:, h:h + 1], in_=tAn, axis=AX.X,
                    op=OP.max, negate=True)
            nmA = perb.tile([P, 1], F32, tag="nma")
            nc.vector.tensor_tensor(out=nmA, in0=nmA2[:, 0:1], in1=nmA2[:, 1:2],
                                    op=OP.min)
            nmrow_ps = ps_sm.tile([1, P], F32, tag="small", name="nmrow_ps")
            nc.tensor.transpose(nmrow_ps, nmA, id128f)
            nmext = perb.tile([1, P + 1], F32, tag="nmext")
            nc.vector.tensor_copy(nmext[0:1, 0:P], nmrow_ps)
            nc.vector.tensor_scalar_add(nmext[0:1, P:P + 1],
                                        nmrow_ps[0:1, P - 1:P], PAD_SLACK)
            # nm_i = min(nmext[i>>3], nmext[(i>>3)+1]), expanded x8 inline
            nm_stage = perb.tile([1, N], F32R, tag="nmstage")
            nc.vector.tensor_tensor(
                out=nm_stage.rearrange("o (g r) -> o g r", r=8),
                in0=rep8(nmext[0:1, 0:P]), in1=rep8(nmext[0:1, 1:P + 1]),
                op=OP.min)
            rt = rhs5_tiles[q]
            nc.sync.dma_start(out=rt[4:5, :], in_=nm_stage)
            nc.sync.dma_start(out=rt[36:37, :], in_=nm_stage)
            nc.gpsimd.dma_start(out=rt[68:69, :], in_=nm_stage)
            nc.gpsimd.dma_start(out=rt[100:101, :], in_=nm_stage)
            st["rhs5"] = rhs5_tiles[q]

        def s3(bi):
            st = state[bi]
            lhsT5, rhs5 = st["lhsT5"], st["rhs5"]
            acc = ps_acc.tile([3, N], F32, tag="acc", name="acc")
            # inner software pipeline: acc(jt-1) issues AFTER tB(jt) so the
            # in-order PE stream never waits on exp(jt) (p-state ramp needs
            # dense PE runs; the serial tB->exp->acc chain cost ~3.6us/jt)
            es = {}

            def emit_acc(jt):
                se = se_all[:, bi, jt, :]
                nc.tensor.matmul(acc[:, 0:H], se, es[jt][:, 0:H],
                                 start=(jt == 0), stop=(jt == NT - 1))
                nc.tensor.matmul(acc[:, H:N], se, es[jt][:, H:N],
                                 start=(jt == 0), stop=(jt == NT - 1))

            for jt in range(NT):
                tB = ps_tb.tile([P, N], F32, tag="tb", name="tB")
                for h in range(2):
                    rg = (2 * jt + h) % 4
                    l5 = lhsT5[32 * rg:32 * rg + 5, jt * P:(jt + 1) * P]
                    r5 = rhs5[32 * rg:32 * rg + 5, h * H:(h + 1) * H]
                    nc.tensor.matmul(tB[:, h * H:(h + 1) * H], l5, r5,
                                     start=True, stop=True,
                                     tile_position=(32 * rg, 0))
                e = epool.tile([P, N], F32R, tag="e", name="e")
                nc.scalar.activation(out=e, in_=tB, func=ACT.Exp)
                es[jt] = e
                if jt >= 1:
                    emit_acc(jt - 1)
            emit_acc(NT - 1)
            # ACT copy: frees acc (bufs=1) promptly -- on DVE this drain
            # queued behind a wave of rowsum work, stalling acc(b+1) on PE
            nd = perb.tile([3, N], F32, tag="nd")
            nc.scalar.copy(out=nd, in_=acc)
            dst = bass.AP(tensor=out3.tensor, offset=out3.offset + bi * N,
                          ap=[[BPC * N, 3], [1, N]])
            nc.sync.dma_start(out=dst, in_=nd)

        # depth-2 software pipeline: batch w's whole prep chain
        # (s1->s1b->s2, ~13us spread over DVE/ACT/PE-small) issues in one
        # wave and hides under s3(w-1)'s ~19us of dense PE work. s3
        # issues FIRST so its cross-engine consumers (nd, next acc) sit
        # at the head of each engine's in-order queue.
        for w in range(BPC + 1):
            if w >= 1:
                s3(w - 1)
            if 2 <= w + 1 < BPC:
                s0(w + 1)
            if w < BPC:
                s1(w)
                s1b(w)
                s2(w)

    nc.compile()
    return nc


_CACHE = {}


def kernel(scores: np.ndarray) -> np.ndarray:
    scores = np.ascontiguousarray(scores, dtype=np.float32)
    assert scores.shape == (B, N)
    if "nc" not in _CACHE:
        _CACHE["nc"] = build_kernel()
    nc = _CACHE["nc"]

    in_maps = [{"scores": scores[c * BPC:(c + 1) * BPC]} for c in range(NCORES)]
    r = run_bass_kernel_spmd(nc, in_maps, core_ids=list(range(NCORES)))
    outs = []
    for c in range(NCORES):
        o3 = r.results[c]["out3"].astype(np.float64)
        outs.append((o3[0] + o3[1]) / o3[2])
    return np.concatenate(outs, axis=0).astype(np.float32)


if __name__ == "__main__":
    x = np.random.randn(B, N).astype(np.float32)
    y = kernel(x)
    print(y.shape, y.dtype)


# revision 55
# speedup vs baseline: 1.2340x; 1.0235x over previous
"""NeuralSort P_hat @ scores kernel for Trainium2 (8 NeuronCores, data-parallel).

Math per batch row s[0:N], N=1024:
  r_j   = sum_k |s_j - s_k|
  a_i   = N + 1 - 2*(i+1) = 1023 - 2i
  t_ij  = a_i s_j - r_j
  out_i = softmax_j(t_i) . s

Design notes (measured on HW; baseline 344us -> ~181us):
- Near-zero DMA traffic: all operand layouts built on-chip with PE
  transposes (v1 spent 152us of sync-queue DIRECT2D triggers + 9k
  4-byte gather descriptors).
- Compute APs may start only at partitions {0,32,64,96}; every
  cross-partition placement here is either a PE transpose (to base 0)
  or a DMA (unrestricted). DMA APs use plain tile slices only --
  multi-level partition-strided APs silently overrun rows.
- tB and tAn matmuls run ROW-TILED (tile_position=(32*rg,0), 32x128 PE
  tiles) 4-way round-robin over SBUF quadrant replicas of the 5 operand
  rows; replicas maintained by small SBUF->SBUF DMAs. Col tiling
  (nonzero dst quadrant) is rejected by this compiler.
- s3's inner loop is software-pipelined (acc(jt-1) issues after
  tB(jt)) so the in-order PE stream never waits on exp; batch-level
  pipeline is depth 2 (whole prep chain hides under prior batch's s3).
- rowsum (ReLU form) split DVE(5)/ACT(3); Pool measured ~6x slower on bulk
  tiles and only runs iota/affine_select + a few replica DMAs.

Per core (BPC=8 batches):
  S0  s_bcast row broadcast (DMA, 128 desc)
  S1  rowsum via ReLU identity: r_j = 2*sum_k relu(s_k - s_j) - S + n*s_j;
      the -S term is row-constant (cancels in softmax, dropped) and n*s_j
      folds into a' = a - n. One fused pass per tile: DVE
      scalar_tensor_tensor(sub, max, sum-accum) / ACT Relu bias-accum
  S1b nr hi/lo splits (col form) + 8 PE transposes -> lhsT5 rows
      (s_hi, nr_hi, s_lo, nr_lo | ones) + quadrant replica DMAs
  S2  anchored row-max bound: f(a)=max_j(a s_j - r_j) is convex; K=2
      row-tiled anchor matmuls at 128 anchors (i=8g) + negate-max
      reduce, PE transpose col->row, Lipschitz pad, neighbor-min with
      x8 stride-0 expansion -> nm row -> rhs5 row 4 (4 quadrant DMAs)
  S3  per jt: K=5 fp32r row-tiled t-matmul [128,1024] (rhs5 =
      (a,1,a,1,nm)), ACT exp -> e (f32r), K=128 reduce matmul -> acc
      (numer_hi, numer_lo, denom); ACT drains acc -> DMA out
Final (hi+lo)/denom division happens on host during unshard (65k flops).

f32r (~12-bit mantissa) operands are hi/lo split where >12 significant
bits matter (s, r); products of split terms are exact and reassemble in
fp32 PSUM. The max-shift M_i needs no lo part: an M error only scales
numer and denom by the same factor, which cancels in the division.
"""

import numpy as np
from contextlib import ExitStack

try:
    import concourse.bass as bass
except ImportError:
    import sys
    sys.path.insert(0, "/opt/trn_rl_repo")
    import concourse.bass as bass
import concourse.tile as tile
from concourse import bacc, mybir
from concourse.bass_utils import run_bass_kernel_spmd

B, N = 64, 1024
NCORES = 8
BPC = B // NCORES      # batches per core
P = 128                # partitions
NT = N // P            # 8 tiles of 128
H = N // 2
F32 = mybir.dt.float32
F32R = mybir.dt.float32r
F16 = mybir.dt.float16
BF16 = mybir.dt.bfloat16
I32 = mybir.dt.int32
AX = mybir.AxisListType
OP = mybir.AluOpType
ACT = mybir.ActivationFunctionType

# rowsum pass assignment per jt: D=DVE 2-pass, A=ACT fused 1-pass.
# Pool is ~6x slower than DVE on bulk tiles (measured) -- bulk work
# never goes there. During pipeline fill (first batches) ACT has no exp
# work yet, so it takes more tiles to shorten the critical path.
RS_STEADY = ("D", "A", "D", "D", "A", "D", "A", "D")
RS_FILL = ("D", "A", "A", "D", "A", "A", "D", "A")
PAD_SLACK = -40.0      # Lipschitz pad for the anchor tail slot


def rep8(base):
    # [1, 128] slice read as [1, 128, 8] with stride-0 inner repeat
    return bass.AP(tensor=base.tensor, offset=base.offset,
                   ap=list(base.ap) + [[0, 8]])


def build_kernel():
    nc = bacc.Bacc("TRN2", target_bir_lowering=False, debug=False)

    scores = nc.dram_tensor("scores", [BPC, N], F32, kind="ExternalInput").ap()
    out3 = nc.dram_tensor("out3", [3, BPC, N], F32, kind="ExternalOutput").ap()

    with tile.TileContext(nc) as tc, ExitStack() as ctx:
        const = ctx.enter_context(tc.tile_pool(name="const", bufs=1))
        perb = ctx.enter_context(tc.tile_pool(name="perb", bufs=2))
        bigs = ctx.enter_context(tc.tile_pool(name="bigs", bufs=2))
        epool = ctx.enter_context(tc.tile_pool(name="epool", bufs=4))
        ps_tb = ctx.enter_context(tc.tile_pool(name="ps_tb", bufs=2, space="PSUM"))
        ps_acc = ctx.enter_context(tc.tile_pool(name="ps_acc", bufs=1, space="PSUM"))
        ps_tan = ctx.enter_context(tc.tile_pool(name="ps_tan", bufs=1, space="PSUM"))
        ps_sm = ctx.enter_context(tc.tile_pool(name="ps_sm", bufs=1, space="PSUM"))

        # critical loads first on the sync FIFO: scores + batch-0/1
        # broadcasts gate the whole fill; setup DMAs queue behind them.
        scores_t = const.tile([BPC, N], F32)
        nc.sync.dma_start(out=scores_t, in_=scores)
        state = {}

        def s0(bi):
            srow = scores[bi:bi + 1, :]
            s_bcast = bigs.tile([P, N], F32, tag="sb")
            nc.sync.dma_start(out=s_bcast, in_=bass.AP(
                tensor=srow.tensor, offset=srow.offset, ap=[[0, P], [1, N]]))
            state[bi] = dict(s_bcast=s_bcast)

        s0(0)
        s0(1)

        # ---- constants (all on-chip; no const inputs) ----
        ones_f = const.tile([P, P], F32)
        nc.vector.memset(ones_f, 1.0)
        id128f = const.tile([P, P], F32)
        nc.gpsimd.affine_select(out=id128f, in_=ones_f, pattern=[[-1, P]],
                                compare_op=OP.is_equal, fill=0.0,
                                base=0, channel_multiplier=1)
        id128r = const.tile([P, P], F32R)
        nc.vector.tensor_copy(id128r, id128f)
        id8f = id128f[0:8, 0:8]

        a_i32 = const.tile([1, N], I32)
        nc.gpsimd.iota(a_i32, pattern=[[-2, N]], base=-1, channel_multiplier=0)
        a_row = const.tile([1, N], F32R)
        nc.vector.tensor_copy(a_row, a_i32)
        anch_i32 = const.tile([1, P], I32)
        nc.gpsimd.iota(anch_i32, pattern=[[-16, P]], base=-1, channel_multiplier=0)
        anch_row = const.tile([1, P], F32R)
        nc.vector.tensor_copy(anch_row, anch_i32)
        ones_row_f = const.tile([1, N], F32)
        nc.vector.memset(ones_row_f, 1.0)
        ones_row = const.tile([1, N], F32R)
        nc.vector.tensor_copy(ones_row, ones_row_f)
        ones_cols = const.tile([P, NT * 8], F32)
        nc.vector.memset(ones_cols, 1.0)
        zeros_t = const.tile([P, N], F32)
        nc.vector.memset(zeros_t, 0.0)

        def quad_rows(t, row):
            pitch = t.ap[0][0]
            return bass.AP(tensor=t.tensor, offset=t.offset + row * pitch,
                           ap=[[32 * pitch, 4], [1, N]])

        def row_at(t, row, n, nrows=1, rowstep=32):
            pitch = t.ap[0][0]
            return bass.AP(tensor=t.tensor, offset=t.offset + row * pitch,
                           ap=[[rowstep * pitch, nrows], [1, n]])

        # a_anch rows (a_{8g}, 1) replicated at SBUF quadrants for
        # row-tiled tAn matmuls
        a_anch = const.tile([98, P], F32R)
        nc.sync.dma_start(out=a_anch[0:1, :], in_=anch_row)
        nc.sync.dma_start(out=a_anch[1:2, :], in_=ones_row[0:1, 0:P])
        nc.sync.dma_start(out=a_anch[32:34, :], in_=a_anch[0:2, :])
        nc.sync.dma_start(out=a_anch[64:66, :], in_=a_anch[0:2, :])
        nc.sync.dma_start(out=a_anch[96:98, :], in_=a_anch[0:2, :])

        # operand tiles, manually rotated 4 deep, with replicas of the
        # 5 operand rows at SBUF quadrants {0,32,64,96} so tB/tAn matmuls
        # can run row-tiled (32x128 PE tiles) 4-way concurrently.
        lhsT5_tiles = [const.tile([101, N], F32R, tag=f"lhsT5_{q}", name=f"lhsT5_{q}")
                       for q in range(4)]
        rhs5_tiles = [const.tile([101, N], F32R, tag=f"rhs5_{q}", name=f"rhs5_{q}")
                      for q in range(4)]

        for q in range(4):
            eng = nc.sync if q % 2 == 0 else nc.gpsimd
            rt = rhs5_tiles[q]
            eng.dma_start(out=rt[0:1, :], in_=a_row)
            eng.dma_start(out=rt[1:2, :], in_=ones_row)
            eng.dma_start(out=rt[2:3, :], in_=a_row)
            eng.dma_start(out=rt[3:4, :], in_=ones_row)
            for rg in (32, 64, 96):
                eng.dma_start(out=rt[rg:rg + 4, :], in_=rt[0:4, :])
            lt = lhsT5_tiles[q]
            for rg in (4, 36, 68, 100):
                eng.dma_start(out=lt[rg:rg + 1, :], in_=ones_row)

        # ---- prologue: column forms via PE transpose ----
        scols_ps = ps_sm.tile([P, NT * 8], F32, tag="small", name="scols_ps")
        for c in range(NT):
            nc.tensor.transpose(scols_ps[:, c * 8:(c + 1) * 8],
                                scores_t[:, c * P:(c + 1) * P], id8f)
        # b-major column forms: X[p, b, jt] so per-batch slices are
        # contiguous [128, NT] (strided micro-ops measured 5-30x slower)
        scols = const.tile([P, 8, NT], F32)
        nc.vector.tensor_copy(scols.rearrange("p b t -> p t b"),
                              scols_ps.rearrange("p (t b) -> p t b", b=8))
        nscols = const.tile([P, 8, NT], F32)
        nc.vector.tensor_scalar_mul(nscols, scols, -1.0)
        shi_cols = const.tile([P, 8, NT], F32R)
        nc.vector.tensor_copy(shi_cols, scols)
        slo_cols = const.tile([P, 8, NT], F32R)
        nc.vector.tensor_sub(slo_cols, scols, shi_cols.bitcast(F32))
        # se_all[p, b, jt, c]: (s_hi, s_lo, 1) columns, contiguous per (b, jt)
        se_all = const.tile([P, 8, NT, 3], F32R)
        se_v = se_all.rearrange("p b t c -> p c b t")  # write views [P,8,NT]
        nc.vector.tensor_copy(se_v[:, 0], shi_cols)
        nc.vector.tensor_copy(se_v[:, 1], slo_cols)
        nc.vector.tensor_copy(se_v[:, 2], ones_cols.rearrange("p (b t) -> p b t", t=NT))

        def s1(bi):
            st = state[bi]
            s_bcast = st["s_bcast"]
            r_cols = perb.tile([P, NT], F32, tag="rcols")
            assign = RS_FILL if bi < 3 else RS_STEADY
            for jt in range(NT):
                if assign[jt] == "A":
                    scr = bigs.tile([P, N], F32, tag="scr_a", bufs=1)
                    nc.scalar.activation(
                        out=scr, in_=s_bcast, func=ACT.Relu,
                        bias=nscols[:, bi, jt:jt + 1], scale=1.0,
                        accum_out=r_cols[:, jt:jt + 1])
                    continue
                # DVE fused 1-pass: A_j = sum_k max(s_k - s_j, 0)
                scr = bigs.tile([P, N], F32, tag="scr_d")
                nc.vector.scalar_tensor_tensor(
                    out=scr, in0=s_bcast, scalar=scols[:, bi, jt:jt + 1],
                    in1=zeros_t, op0=OP.subtract, op1=OP.max,
                    accum_out=r_cols[:, jt:jt + 1])
            st["r_cols"] = r_cols

        def s1b(bi):
            st = state[bi]
            r_cols = st["r_cols"]
            q = bi % 4
            # contiguous computes first, then interleave-casts into cols4
            nrhi_c = perb.tile([P, NT], F32R, tag="nrhi")
            nc.vector.tensor_scalar_mul(nrhi_c, r_cols, -2.0)
            nrlo_c = perb.tile([P, NT], F32R, tag="nrlo")
            nc.vector.scalar_tensor_tensor(
                out=nrlo_c, in0=r_cols, scalar=-2.0,
                in1=nrhi_c.bitcast(F32), op0=OP.mult, op1=OP.subtract)
            # cols4[p, jt, c] = (s_hi, nr_hi, s_lo, nr_lo)
            cols4 = perb.tile([P, NT, 4], F32R, tag="cols4")
            nc.vector.tensor_copy(cols4[:, :, 0], shi_cols[:, bi, :])
            nc.vector.tensor_copy(cols4[:, :, 1], nrhi_c)
            nc.vector.tensor_copy(cols4[:, :, 2], slo_cols[:, bi, :])
            nc.vector.tensor_copy(cols4[:, :, 3], nrlo_c)
            lhsT5 = lhsT5_tiles[q]
            for half in range(2):
                l5h = ps_sm.tile([4, H], F32R, tag="small", name="l5h")
                for j in range(4):
                    jt = half * 4 + j
                    nc.tensor.transpose(l5h[:, j * P:(j + 1) * P],
                                        cols4[:, jt, :], id128r)
                nc.vector.tensor_copy(lhsT5[0:4, half * H:(half + 1) * H], l5h)
            nc.gpsimd.dma_start(out=lhsT5[32:36, :], in_=lhsT5[0:4, :])
            nc.gpsimd.dma_start(out=lhsT5[64:68, :], in_=lhsT5[0:4, :])
            nc.gpsimd.dma_start(out=lhsT5[96:100, :], in_=lhsT5[0:4, :])
            st["lhsT5"] = lhsT5

        def s2(bi):
            st = state[bi]
            lhsT5 = st["lhsT5"]
            q = bi % 4
            # f(a) = max_j (a s_j - r_j) is convex in a; anchors at i=8g.
            # bound M_i = max(f(a_g), f(a_{g+1})) >= f(a_i), slack <= ~40.
            nmA2 = perb.tile([P, 2], F32, tag="nma2")
            for h in range(2):
                tAn = ps_tan.tile([P, H], F32, tag="tan", name="tAn")
                nc.tensor.matmul(tAn, a_anch[32 * h:32 * h + 2, :],
                                 lhsT5[32 * h:32 * h + 2, h * H:(h + 1) * H],
                                 start=True, stop=True,
                                 tile_position=(32 * h, 0))
                nc.vector.tensor_reduce(
                    out=nmA2[:, h:h + 1], in_=tAn, axis=AX.X,
                    op=OP.max, negate=True)
            nmA = perb.tile([P, 1], F32, tag="nma")
            nc.vector.tensor_tensor(out=nmA, in0=nmA2[:, 0:1], in1=nmA2[:, 1:2],
                                    op=OP.min)
            nmrow_ps = ps_sm.tile([1, P], F32, tag="small", name="nmrow_ps")
            nc.tensor.transpose(nmrow_ps, nmA, id128f)
            nmext = perb.tile([1, P + 1], F32, tag="nmext")
            nc.vector.tensor_copy(nmext[0:1, 0:P], nmrow_ps)
            nc.vector.tensor_scalar_add(nmext[0:1, P:P + 1],
                                        nmrow_ps[0:1, P - 1:P], PAD_SLACK)
            # nm_i = min(nmext[i>>3], nmext[(i>>3)+1]), expanded x8 inline
            nm_stage = perb.tile([1, N], F32R, tag="nmstage")
            nc.vector.tensor_tensor(
                out=nm_stage.rearrange("o (g r) -> o g r", r=8),
                in0=rep8(nmext[0:1, 0:P]), in1=rep8(nmext[0:1, 1:P + 1]),
                op=OP.min)
            rt = rhs5_tiles[q]
            nc.sync.dma_start(out=rt[4:5, :], in_=nm_stage)
            nc.sync.dma_start(out=rt[36:37, :], in_=nm_stage)
            nc.gpsimd.dma_start(out=rt[68:69, :], in_=nm_stage)
            nc.gpsimd.dma_start(out=rt[100:101, :], in_=nm_stage)
            st["rhs5"] = rhs5_tiles[q]

        def s3(bi):
            st = state[bi]
            lhsT5, rhs5 = st["lhsT5"], st["rhs5"]
            acc = ps_acc.tile([3, N], F32, tag="acc", name="acc")
            # inner software pipeline: acc(jt-1) issues AFTER tB(jt) so the
            # in-order PE stream never waits on exp(jt) (p-state ramp needs
            # dense PE runs; the serial tB->exp->acc chain cost ~3.6us/jt)
            es = {}

            def emit_acc(jt):
                se = se_all[:, bi, jt, :]
                nc.tensor.matmul(acc[:, 0:H], se, es[jt][:, 0:H],
                                 start=(jt == 0), stop=(jt == NT - 1))
                nc.tensor.matmul(acc[:, H:N], se, es[jt][:, H:N],
                                 start=(jt == 0), stop=(jt == NT - 1))

            for jt in range(NT):
                tB = ps_tb.tile([P, N], F32, tag="tb", name="tB")
                for h in range(2):
                    rg = (2 * jt + h) % 4
                    l5 = lhsT5[32 * rg:32 * rg + 5, jt * P:(jt + 1) * P]
                    r5 = rhs5[32 * rg:32 * rg + 5, h * H:(h + 1) * H]
                    nc.tensor.matmul(tB[:, h * H:(h + 1) * H], l5, r5,
                                     start=True, stop=True,
                                     tile_position=(32 * rg, 0))
                e = epool.tile([P, N], F32R, tag="e", name="e")
                nc.scalar.activation(out=e, in_=tB, func=ACT.Exp)
                es[jt] = e
                if jt >= 1:
                    emit_acc(jt - 1)
            emit_acc(NT - 1)
            # ACT copy: frees acc (bufs=1) promptly -- on DVE this drain
            # queued behind a wave of rowsum work, stalling acc(b+1) on PE
            nd = perb.tile([3, N], F32, tag="nd")
            nc.scalar.copy(out=nd, in_=acc)
            dst = bass.AP(tensor=out3.tensor, offset=out3.offset + bi * N,
                          ap=[[BPC * N, 3], [1, N]])
            nc.sync.dma_start(out=dst, in_=nd)

        # depth-2 software pipeline: batch w's whole prep chain
        # (s1->s1b->s2, ~13us spread over DVE/ACT/PE-small) issues in one
        # wave and hides under s3(w-1)'s ~19us of dense PE work. s3
        # issues FIRST so its cross-engine consumers (nd, next acc) sit
        # at the head of each engine's in-order queue.
        for w in range(BPC + 1):
            if w >= 1:
                s3(w - 1)
            if 2 <= w + 1 < BPC:
                s0(w + 1)
            if w < BPC:
                s1(w)
                s1b(w)
                s2(w)

    nc.compile()
    return nc


_CACHE = {}


def kernel(scores: np.ndarray) -> np.ndarray:
    scores = np.ascontiguousarray(scores, dtype=np.float32)
    assert scores.shape == (B, N)
    if "nc" not in _CACHE:
        _CACHE["nc"] = build_kernel()
    nc = _CACHE["nc"]

    in_maps = [{"scores": scores[c * BPC:(c + 1) * BPC]} for c in range(NCORES)]
    r = run_bass_kernel_spmd(nc, in_maps, core_ids=list(range(NCORES)))
    outs = []
    for c in range(NCORES):
        o3 = r.results[c]["out3"].astype(np.float64)
        outs.append((o3[0] + o3[1]) / o3[2])
    return np.concatenate(outs, axis=0).astype(np.float32)


if __name__ == "__main__":
    x = np.random.randn(B, N).astype(np.float32)
    y = kernel(x)
    print(y.shape, y.dtype)
